# revision 9
# baseline (speedup 1.0000x reference)
"""Trainium2 Bass kernel for nn_Attention_81776177315877.

Separable-conv attention block (CMT/PVT style):
  x (B=8, 3136, 256) -> q/k/v = sepconv(dw3x3+BN+pw1x1, k/v stride 2)
  -> 8-head attention (d=32) -> proj.

Sharding: data-parallel over batch, core b <- batch b. No collectives.

Device strategy (per core, all f32, channel-major layouts):
  - fold BN+depthwise taps into the pointwise weights on host: the whole
    sepconv = sum over 9 taps of (W''_tap @ x_shifted) + const, computed as
    PSUM-accumulated matmuls at full K=128 utilization over a zero-padded
    channel-major image.
  - attention: S^T (keys on partitions) via 4-head tile_position row-packing
    (contraction d=32), exp on ScalarE (no max-subtraction: |S| <~ 2 here,
    mathematically identical softmax), O^T and the softmax denominator via
    col-packed K=112 matmuls, normalization on VectorE with the denominator
    replicated across each head's 32 partitions.
"""

import sys

sys.path.insert(0, "/opt/trn_rl_repo")

import numpy as np
import ml_dtypes

import concourse.bass as bass
import concourse.bacc as bacc
import concourse.mybir as mybir
import concourse.tile as tile
from concourse.bass_utils import run_bass_kernel_spmd
from concourse.masks import make_identity

FP = mybir.dt.float32
BF = mybir.dt.bfloat16
AF = mybir.ActivationFunctionType

C = 256
HEADS = 8
D = 32
HH = 56
N = HH * HH          # 3136 query tokens
HK = 28
NK = HK * HK         # 784 key tokens
PADW = HH + 2        # 58
EPS = 1e-5
SCALE = D ** -0.5

IC_CH = 8            # query rows per chunk -> 448 free
IC_F = IC_CH * HH    # 448
N_IC = HH // IC_CH   # 7
KC_CH = 14           # k/v output rows per chunk -> 392 free
KC_F = KC_CH * HK    # 392
N_KC = HK // KC_CH   # 2
JT = 112             # key tile (partitions) for attention
N_JT = NK // JT      # 7

_CACHED = {}


def _build_nc():
    nc = bacc.Bacc("TRN2", target_bir_lowering=False, debug=False, num_devices=8)

    x_d = nc.dram_tensor("x", [N, C], BF, kind="ExternalInput")
    scr_d = nc.dram_tensor("scr", [C, N], BF)
    w9t_d = {}
    const_d = {}
    for p in ("q", "k", "v"):
        w9t_d[p] = nc.dram_tensor(f"{p}_w9t", [9, C, C], BF, kind="ExternalInput")
        const_d[p] = nc.dram_tensor(f"{p}_const", [C, 1], FP, kind="ExternalInput")
    pwt_d = nc.dram_tensor("proj_wt", [C, C], BF, kind="ExternalInput")
    pb_d = nc.dram_tensor("proj_bv", [C, 1], FP, kind="ExternalInput")
    out_d = nc.dram_tensor("out", [N, C], FP, kind="ExternalOutput")

    with tile.TileContext(nc) as tc:
        with (
            tc.tile_pool(name="persist", bufs=1) as pp,
            tc.tile_pool(name="big", bufs=4) as bigp,
            tc.tile_pool(name="xt", bufs=3) as xtp,
            tc.tile_pool(name="wt", bufs=2) as wtp,
            tc.tile_pool(name="ep", bufs=4) as ep,
            tc.tile_pool(name="rp", bufs=2) as rp,
            tc.tile_pool(name="ps", bufs=2, space="PSUM") as psp,
            tc.tile_pool(name="pss", bufs=2, space="PSUM") as pss,
            tc.tile_pool(name="pso", bufs=1, space="PSUM") as pso,
            tc.tile_pool(name="psd", bufs=1, space="PSUM") as psd,
        ):
            ident = pp.tile([128, 128], FP, tag="ident", name="ident")
            make_identity(nc, ident[:])
            ones = pp.tile([128, 32], BF, tag="ones", name="ones")
            nc.gpsimd.memset(ones[:], 1.0)

            # ---- load folded weights ----
            w9t = {}
            consts = {}
            for p in ("q", "k", "v"):
                w9t[p] = [pp.tile([128, 9, C], BF, tag=f"w9t_{p}{cb}", name=f"w9t_{p}{cb}") for cb in range(2)]
                for cb in range(2):
                    nc.sync.dma_start(
                        w9t[p][cb][:],
                        w9t_d[p][:, cb * 128:(cb + 1) * 128, :].rearrange("t c o -> c t o"),
                    )
                consts[p] = [pp.tile([128, 1], FP, tag=f"const_{p}{cb}", name=f"const_{p}{cb}") for cb in range(2)]
                for cb in range(2):
                    nc.sync.dma_start(consts[p][cb][:], const_d[p][cb * 128:(cb + 1) * 128, :])
            proj_wt = [pp.tile([128, C], BF, tag=f"pwt{cb}", name=f"pwt{cb}") for cb in range(2)]
            proj_b = [pp.tile([128, 1], FP, tag=f"pb{cb}", name=f"pb{cb}") for cb in range(2)]
            for cb in range(2):
                nc.sync.dma_start(proj_wt[cb][:], pwt_d[cb * 128:(cb + 1) * 128, :])
                nc.sync.dma_start(proj_b[cb][:], pb_d[cb * 128:(cb + 1) * 128, :])

            # ---- phase 0: x -> channel-major padded image ----
            x_pad = [pp.tile([128, PADW, PADW], BF, tag=f"xpad{cb}", name=f"xpad{cb}") for cb in range(2)]
            for cb in range(2):
                nc.gpsimd.memset(x_pad[cb][:], 0.0)
            x_cm = [bigp.tile([128, N], BF, tag="big", name="big") for _ in range(2)]
            n_tt = (N + 127) // 128  # 25
            for cb in range(2):
                nc.sync.dma_start_transpose(
                    x_cm[cb][:], x_d[:, cb * 128:(cb + 1) * 128]
                )
            for cb in range(2):
                nc.vector.tensor_copy(
                    x_pad[cb][:, 1:57, 1:57],
                    x_cm[cb][:].rearrange("p (h w) -> p h w", w=HH),
                )

            # ---- conv helper: sepconv as 9 shifted matmuls ----
            def conv_chunk(p, dst_tiles, stride, ch_rows, wo, ch_idx):
                # output rows [ch_idx*ch_rows, ...), all wo cols
                fsz = ch_rows * wo
                for cbo in range(2):
                    cps = psp.tile([128, 448], FP, tag="ps", name="ps")
                    nmm = 0
                    for cbi in range(2):
                        for tap in range(9):
                            dh, dw = tap // 3 - 1, tap % 3 - 1
                            r0 = 1 + stride * ch_idx * ch_rows + dh
                            c0 = 1 + dw
                            if stride == 1:
                                rhs = x_pad[cbi][:, r0:r0 + ch_rows, c0:c0 + wo]
                            else:
                                xp2 = x_pad[cbi][:].rearrange(
                                    "p (ho a) (wv b) -> p ho a wv b", a=2, b=2
                                )
                                rhs = xp2[
                                    :,
                                    r0 // 2: r0 // 2 + ch_rows,
                                    r0 % 2,
                                    c0 // 2: c0 // 2 + wo,
                                    c0 % 2,
                                ]
                            nc.tensor.matmul(
                                cps[:, :fsz],
                                lhsT=(w9t[p][cbi][:, tap, cbo * 128:(cbo + 1) * 128]),
                                rhs=(rhs),
                                start=(nmm == 0),
                                stop=(nmm == 17),
                            )
                            nmm += 1
                    nc.vector.tensor_scalar_add(
                        dst_tiles[cbo][:, ch_idx * fsz:(ch_idx + 1) * fsz],
                        cps[:, :fsz],
                        consts[p][cbo],
                    )

            # ---- k, v convs (full), v transpose to token-major ----
            k_cm = [pp.tile([128, NK], BF, tag=f"kcm{cb}", name=f"kcm{cb}") for cb in range(2)]
            v_cm = [pp.tile([128, NK], FP, tag=f"vcm{cb}", name=f"vcm{cb}") for cb in range(2)]
            for ch in range(N_KC):
                conv_chunk("k", k_cm, 2, KC_CH, HK, ch)
            for ch in range(N_KC):
                conv_chunk("v", v_cm, 2, KC_CH, HK, ch)
            v_tm = pp.tile([128, N_JT, C], BF, tag="vtm", name="vtm")
            for jt in range(N_JT):
                for cb in range(2):
                    tp = psp.tile([128, 448], FP, tag="ps", name="ps")
                    nc.tensor.transpose(
                        tp[:JT, :128],
                        v_cm[cb][:, jt * JT:(jt + 1) * JT],
                        ident[:],
                    )
                    nc.vector.tensor_copy(
                        v_tm[:JT, jt, cb * 128:(cb + 1) * 128], tp[:JT, :128]
                    )

            # ---- phase 2: q conv chunk + attention, interleaved ----
            q_cm = [pp.tile([128, N], BF, tag=f"qcm{cb}", name=f"qcm{cb}") for cb in range(2)]
            o_cm = [bigp.tile([128, N], BF, tag="big", name="big") for _ in range(2)]
            for ic in range(N_IC):
                conv_chunk("q", q_cm, 1, IC_CH, HH, ic)
                for hg in range(2):
                    o_ps = pso.tile([128, IC_F], FP, tag="o", name="o")
                    d_ps = psd.tile([128, IC_F], FP, tag="d", name="d")
                    for jt in range(N_JT):
                        for hp in range(2):
                            s2 = pss.tile([128, 2, 512], FP, tag="s", name="s")
                            for h2 in range(2):
                                hh = hp * 2 + h2
                                nc.tensor.matmul(
                                    s2[:JT, h2, :IC_F],
                                    lhsT=(k_cm[hg][hh * 32:(hh + 1) * 32, jt * JT:(jt + 1) * JT]),
                                    rhs=(q_cm[hg][hh * 32:(hh + 1) * 32, ic * IC_F:(ic + 1) * IC_F]),
                                    start=True,
                                    stop=True,
                                    tile_position=(32 * hh, 0),
                                )
                            e2 = ep.tile([128, 2, IC_F], BF, tag="e", name="e")
                            nc.scalar.activation(
                                e2[:JT, :, :], s2[:JT, :, :IC_F], AF.Exp, scale=SCALE
                            )
                            for h2 in range(2):
                                hh = hp * 2 + h2
                                nc.tensor.matmul(
                                    o_ps[hh * 32:(hh + 1) * 32, :],
                                    lhsT=(v_tm[:JT, jt, hg * 128 + hh * 32: hg * 128 + (hh + 1) * 32]),
                                    rhs=(e2[:JT, h2, :]),
                                    start=(jt == 0),
                                    stop=(jt == N_JT - 1),
                                    tile_position=(0, 32 * hh),
                                    skip_group_check=True,
                                )
                                nc.tensor.matmul(
                                    d_ps[hh * 32:(hh + 1) * 32, :],
                                    lhsT=(ones[:JT, :]),
                                    rhs=(e2[:JT, h2, :]),
                                    start=(jt == 0),
                                    stop=(jt == N_JT - 1),
                                    tile_position=(0, 32 * hh),
                                    skip_group_check=True,
                                )
                    r_t = rp.tile([128, IC_F], FP, tag="r", name="r")
                    nc.vector.reciprocal_approx_fast(r_t[:], d_ps[:])
                    nc.vector.tensor_mul(
                        o_cm[hg][:, ic * IC_F:(ic + 1) * IC_F], o_ps[:], r_t[:]
                    )

            # ---- proj ----
            out_cm = [bigp.tile([128, N], BF, tag="big", name="big") for _ in range(2)]
            for ob in range(2):
                for ic in range(N_IC):
                    cps = psp.tile([128, 448], FP, tag="ps", name="ps")
                    for cb in range(2):
                        nc.tensor.matmul(
                            cps[:, :IC_F],
                            lhsT=(proj_wt[cb][:, ob * 128:(ob + 1) * 128]),
                            rhs=(o_cm[cb][:, ic * IC_F:(ic + 1) * IC_F]),
                            start=(cb == 0),
                            stop=(cb == 1),
                        )
                    nc.vector.tensor_scalar_add(
                        out_cm[ob][:, ic * IC_F:(ic + 1) * IC_F],
                        cps[:, :IC_F],
                        proj_b[ob],
                    )

            # ---- output transpose back to token-major + store ----
            for cb in range(2):
                nc.sync.dma_start(scr_d[cb * 128:(cb + 1) * 128, :], out_cm[cb][:])
            for ti in range(n_tt):
                st = min(ti * 128, N - 128)  # overlap the ragged tail
                otb = xtp.tile([128, C], BF, tag="xtb", name="xtb")
                nc.sync.dma_start_transpose(otb[:], scr_d[:, st: st + 128])
                ot = xtp.tile([128, C], FP, tag="xt", name="xt")
                nc.vector.tensor_copy(ot[:], otb[:])
                nc.sync.dma_start(out_d[st: st + 128, :], ot[:])

    nc.compile()
    return nc


def _fold_weights(inp, p):
    scale = inp[f"{p}_bn_g"] / np.sqrt(inp[f"{p}_bn_v"] + EPS)
    shift = inp[f"{p}_bn_b"] - inp[f"{p}_bn_m"] * scale
    w2 = inp[f"{p}_pw_w"] * scale[None, :]          # (o, c)
    w9 = inp[f"{p}_dw_w"].reshape(C, 9)             # (c, tap)
    w9t = np.ascontiguousarray(
        w2.T[None, :, :] * w9.T[:, :, None]          # (tap, c, o)
    ).astype(ml_dtypes.bfloat16)
    const = (
        inp[f"{p}_pw_w"] @ (scale * inp[f"{p}_dw_b"] + shift) + inp[f"{p}_pw_b"]
    ).astype(np.float32)
    return w9t, const.reshape(C, 1)


def kernel(**inputs):
    inp = {k: np.asarray(v) for k, v in inputs.items()}
    x = inp["x"].astype(np.float32)          # (8, 3136, 256)
    B = x.shape[0]

    if "nc" not in _CACHED:
        _CACHED["nc"] = _build_nc()
    nc = _CACHED["nc"]

    common = {}
    for p in ("q", "k", "v"):
        w9t, const = _fold_weights(inp, p)
        common[f"{p}_w9t"] = w9t
        common[f"{p}_const"] = const
    common["proj_wt"] = np.ascontiguousarray(inp["proj_w"].T).astype(ml_dtypes.bfloat16)
    common["proj_bv"] = inp["proj_b"].reshape(C, 1).astype(np.float32)

    xb = x.astype(ml_dtypes.bfloat16)
    in_maps = [dict(common, x=np.ascontiguousarray(xb[b])) for b in range(B)]
    res = run_bass_kernel_spmd(nc, in_maps, list(range(B)))
    out = np.stack([res.results[b]["out"] for b in range(B)], axis=0)
    return out.astype(np.float32)


# revision 10
# speedup vs baseline: 1.2200x; 1.2200x over previous
"""Trainium2 Bass kernel for nn_Attention_81776177315877.

Separable-conv attention block (CMT/PVT style):
  x (B=8, 3136, 256) -> q/k/v = sepconv(dw3x3+BN+pw1x1, k/v stride 2)
  -> 8-head attention (d=32) -> proj.

Sharding: data-parallel over batch, core b <- batch b. No collectives.

Device strategy (per core, all f32, channel-major layouts):
  - fold BN+depthwise taps into the pointwise weights on host: the whole
    sepconv = sum over 9 taps of (W''_tap @ x_shifted) + const, computed as
    PSUM-accumulated matmuls at full K=128 utilization over a zero-padded
    channel-major image.
  - attention: S^T (keys on partitions) via 4-head tile_position row-packing
    (contraction d=32), exp on ScalarE (no max-subtraction: |S| <~ 2 here,
    mathematically identical softmax), O^T and the softmax denominator via
    col-packed K=112 matmuls, normalization on VectorE with the denominator
    replicated across each head's 32 partitions.
"""

import sys

sys.path.insert(0, "/opt/trn_rl_repo")

import numpy as np
import ml_dtypes

import concourse.bass as bass
import concourse.bacc as bacc
import concourse.mybir as mybir
import concourse.tile as tile
from concourse.bass_utils import run_bass_kernel_spmd
from concourse.masks import make_identity

FP = mybir.dt.float32
BF = mybir.dt.bfloat16
AF = mybir.ActivationFunctionType

C = 256
HEADS = 8
D = 32
HH = 56
N = HH * HH          # 3136 query tokens
HK = 28
NK = HK * HK         # 784 key tokens
PADW = HH + 2        # 58
EPS = 1e-5
SCALE = D ** -0.5

IC_CH = 8            # query rows per chunk -> 448 free
IC_F = IC_CH * HH    # 448
N_IC = HH // IC_CH   # 7
KC_CH = 14           # k/v output rows per chunk -> 392 free
KC_F = KC_CH * HK    # 392
N_KC = HK // KC_CH   # 2
JT = 112             # key tile (partitions) for attention
N_JT = NK // JT      # 7

_CACHED = {}


def _build_nc():
    nc = bacc.Bacc("TRN2", target_bir_lowering=False, debug=False, num_devices=8)

    x_d = nc.dram_tensor("x", [N, C], BF, kind="ExternalInput")
    scr_d = nc.dram_tensor("scr", [C, N], BF)
    w9t_d = {}
    const_d = {}
    for p in ("q", "k", "v"):
        w9t_d[p] = nc.dram_tensor(f"{p}_w9t", [9, C, C], BF, kind="ExternalInput")
        const_d[p] = nc.dram_tensor(f"{p}_const", [C, 1], FP, kind="ExternalInput")
    pwt_d = nc.dram_tensor("proj_wt", [C, C], BF, kind="ExternalInput")
    pb_d = nc.dram_tensor("proj_bv", [C, 1], FP, kind="ExternalInput")
    out_d = nc.dram_tensor("out", [N, C], FP, kind="ExternalOutput")

    with tile.TileContext(nc) as tc:
        with (
            tc.tile_pool(name="persist", bufs=1) as pp,
            tc.tile_pool(name="big", bufs=4) as bigp,
            tc.tile_pool(name="xt", bufs=3) as xtp,
            tc.tile_pool(name="wt", bufs=2) as wtp,
            tc.tile_pool(name="ep", bufs=4) as ep,
            tc.tile_pool(name="rp", bufs=2) as rp,
            tc.tile_pool(name="ps", bufs=2, space="PSUM") as psp,
            tc.tile_pool(name="pss", bufs=1, space="PSUM") as pss,
            tc.tile_pool(name="pso", bufs=1, space="PSUM") as pso,
            tc.tile_pool(name="psd", bufs=1, space="PSUM") as psd,
        ):
            ident = pp.tile([128, 128], FP, tag="ident", name="ident")
            make_identity(nc, ident[:])
            ones = pp.tile([128, 32], BF, tag="ones", name="ones")
            nc.gpsimd.memset(ones[:], 1.0)

            # ---- load folded weights ----
            w9t = {}
            consts = {}
            for p in ("q", "k", "v"):
                w9t[p] = [pp.tile([128, 9, C], BF, tag=f"w9t_{p}{cb}", name=f"w9t_{p}{cb}") for cb in range(2)]
                for cb in range(2):
                    nc.sync.dma_start(
                        w9t[p][cb][:],
                        w9t_d[p][:, cb * 128:(cb + 1) * 128, :].rearrange("t c o -> c t o"),
                    )
                consts[p] = [pp.tile([128, 1], FP, tag=f"const_{p}{cb}", name=f"const_{p}{cb}") for cb in range(2)]
                for cb in range(2):
                    nc.sync.dma_start(consts[p][cb][:], const_d[p][cb * 128:(cb + 1) * 128, :])
            proj_wt = [pp.tile([128, C], BF, tag=f"pwt{cb}", name=f"pwt{cb}") for cb in range(2)]
            proj_b = [pp.tile([128, 1], FP, tag=f"pb{cb}", name=f"pb{cb}") for cb in range(2)]
            for cb in range(2):
                nc.sync.dma_start(proj_wt[cb][:], pwt_d[cb * 128:(cb + 1) * 128, :])
                nc.sync.dma_start(proj_b[cb][:], pb_d[cb * 128:(cb + 1) * 128, :])

            # ---- phase 0: x -> channel-major padded image ----
            x_pad = [pp.tile([128, PADW, PADW], BF, tag=f"xpad{cb}", name=f"xpad{cb}") for cb in range(2)]
            for cb in range(2):
                nc.gpsimd.memset(x_pad[cb][:], 0.0)
            x_cm = [bigp.tile([128, N], BF, tag="big", name="big") for _ in range(2)]
            n_tt = (N + 127) // 128  # 25
            for cb in range(2):
                nc.sync.dma_start_transpose(
                    x_cm[cb][:], x_d[:, cb * 128:(cb + 1) * 128]
                )
            for cb in range(2):
                nc.vector.tensor_copy(
                    x_pad[cb][:, 1:57, 1:57],
                    x_cm[cb][:].rearrange("p (h w) -> p h w", w=HH),
                )

            # ---- conv helper: sepconv as 9 shifted matmuls ----
            def conv_chunk(p, dst_tiles, stride, ch_rows, wo, ch_idx):
                # output rows [ch_idx*ch_rows, ...), all wo cols
                fsz = ch_rows * wo
                for cbo in range(2):
                    cps = psp.tile([128, 448], FP, tag="ps", name="ps")
                    nmm = 0
                    for cbi in range(2):
                        for tap in range(9):
                            dh, dw = tap // 3 - 1, tap % 3 - 1
                            r0 = 1 + stride * ch_idx * ch_rows + dh
                            c0 = 1 + dw
                            if stride == 1:
                                rhs = x_pad[cbi][:, r0:r0 + ch_rows, c0:c0 + wo]
                            else:
                                xp2 = x_pad[cbi][:].rearrange(
                                    "p (ho a) (wv b) -> p ho a wv b", a=2, b=2
                                )
                                rhs = xp2[
                                    :,
                                    r0 // 2: r0 // 2 + ch_rows,
                                    r0 % 2,
                                    c0 // 2: c0 // 2 + wo,
                                    c0 % 2,
                                ]
                            nc.tensor.matmul(
                                cps[:, :fsz],
                                lhsT=(w9t[p][cbi][:, tap, cbo * 128:(cbo + 1) * 128]),
                                rhs=(rhs),
                                start=(nmm == 0),
                                stop=(nmm == 17),
                            )
                            nmm += 1
                    nc.vector.tensor_scalar_add(
                        dst_tiles[cbo][:, ch_idx * fsz:(ch_idx + 1) * fsz],
                        cps[:, :fsz],
                        consts[p][cbo],
                    )

            # ---- k, v convs (full), v transpose to token-major ----
            k_cm = [pp.tile([128, NK], BF, tag=f"kcm{cb}", name=f"kcm{cb}") for cb in range(2)]
            v_cm = [pp.tile([128, NK], FP, tag=f"vcm{cb}", name=f"vcm{cb}") for cb in range(2)]
            for ch in range(N_KC):
                conv_chunk("k", k_cm, 2, KC_CH, HK, ch)
            for ch in range(N_KC):
                conv_chunk("v", v_cm, 2, KC_CH, HK, ch)
            v_tm = pp.tile([128, N_JT, C], BF, tag="vtm", name="vtm")
            for jt in range(N_JT):
                for cb in range(2):
                    tp = psp.tile([128, 448], FP, tag="ps", name="ps")
                    nc.tensor.transpose(
                        tp[:JT, :128],
                        v_cm[cb][:, jt * JT:(jt + 1) * JT],
                        ident[:],
                    )
                    nc.vector.tensor_copy(
                        v_tm[:JT, jt, cb * 128:(cb + 1) * 128], tp[:JT, :128]
                    )

            # ---- phase 2: q conv chunk + attention, interleaved ----
            q_cm = [pp.tile([128, N], BF, tag=f"qcm{cb}", name=f"qcm{cb}") for cb in range(2)]
            o_cm = [bigp.tile([128, N], BF, tag="big", name="big") for _ in range(2)]
            for ic in range(N_IC):
                conv_chunk("q", q_cm, 1, IC_CH, HH, ic)
                for hg in range(2):
                    o_ps = pso.tile([128, IC_F], FP, tag="o", name="o")
                    d_ps = psd.tile([128, IC_F], FP, tag="d", name="d")

                    def s_mm(jt):
                        s4 = pss.tile([128, 4, 512], FP, tag="s", name="s")
                        for hh in range(4):
                            nc.tensor.matmul(
                                s4[:JT, hh, :IC_F],
                                lhsT=(k_cm[hg][hh * 32:(hh + 1) * 32, jt * JT:(jt + 1) * JT]),
                                rhs=(q_cm[hg][hh * 32:(hh + 1) * 32, ic * IC_F:(ic + 1) * IC_F]),
                                start=True,
                                stop=True,
                                tile_position=(32 * hh, 0),
                            )
                        return s4

                    s4 = s_mm(0)
                    for jt in range(N_JT):
                        e4 = ep.tile([128, 4, IC_F], BF, tag="e", name="e")
                        nc.scalar.activation(
                            e4[:JT, :, :], s4[:JT, :, :IC_F], AF.Exp, scale=SCALE
                        )
                        if jt + 1 < N_JT:
                            s4 = s_mm(jt + 1)
                        for hh in range(4):
                            nc.tensor.matmul(
                                o_ps[hh * 32:(hh + 1) * 32, :],
                                lhsT=(v_tm[:JT, jt, hg * 128 + hh * 32: hg * 128 + (hh + 1) * 32]),
                                rhs=(e4[:JT, hh, :]),
                                start=(jt == 0),
                                stop=(jt == N_JT - 1),
                                tile_position=(0, 32 * hh),
                                skip_group_check=True,
                            )
                            nc.tensor.matmul(
                                d_ps[hh * 32:(hh + 1) * 32, :],
                                lhsT=(ones[:JT, :]),
                                rhs=(e4[:JT, h2, :]) if False else (e4[:JT, hh, :]),
                                start=(jt == 0),
                                stop=(jt == N_JT - 1),
                                tile_position=(0, 32 * hh),
                                skip_group_check=True,
                            )
                    r_t = rp.tile([128, IC_F], FP, tag="r", name="r")
                    nc.vector.reciprocal_approx_fast(r_t[:], d_ps[:])
                    nc.vector.tensor_mul(
                        o_cm[hg][:, ic * IC_F:(ic + 1) * IC_F], o_ps[:], r_t[:]
                    )

            # ---- proj ----
            out_cm = [bigp.tile([128, N], BF, tag="big", name="big") for _ in range(2)]
            for ob in range(2):
                for ic in range(N_IC):
                    cps = psp.tile([128, 448], FP, tag="ps", name="ps")
                    for cb in range(2):
                        nc.tensor.matmul(
                            cps[:, :IC_F],
                            lhsT=(proj_wt[cb][:, ob * 128:(ob + 1) * 128]),
                            rhs=(o_cm[cb][:, ic * IC_F:(ic + 1) * IC_F]),
                            start=(cb == 0),
                            stop=(cb == 1),
                        )
                    nc.vector.tensor_scalar_add(
                        out_cm[ob][:, ic * IC_F:(ic + 1) * IC_F],
                        cps[:, :IC_F],
                        proj_b[ob],
                    )

            # ---- output transpose back to token-major + store ----
            for cb in range(2):
                nc.sync.dma_start(scr_d[cb * 128:(cb + 1) * 128, :], out_cm[cb][:])
            for ti in range(n_tt):
                st = min(ti * 128, N - 128)  # overlap the ragged tail
                otb = xtp.tile([128, C], BF, tag="xtb", name="xtb")
                nc.sync.dma_start_transpose(otb[:], scr_d[:, st: st + 128])
                ot = xtp.tile([128, C], FP, tag="xt", name="xt")
                nc.vector.tensor_copy(ot[:], otb[:])
                nc.sync.dma_start(out_d[st: st + 128, :], ot[:])

    nc.compile()
    return nc


def _fold_weights(inp, p):
    scale = inp[f"{p}_bn_g"] / np.sqrt(inp[f"{p}_bn_v"] + EPS)
    shift = inp[f"{p}_bn_b"] - inp[f"{p}_bn_m"] * scale
    w2 = inp[f"{p}_pw_w"] * scale[None, :]          # (o, c)
    w9 = inp[f"{p}_dw_w"].reshape(C, 9)             # (c, tap)
    w9t = np.ascontiguousarray(
        w2.T[None, :, :] * w9.T[:, :, None]          # (tap, c, o)
    ).astype(ml_dtypes.bfloat16)
    const = (
        inp[f"{p}_pw_w"] @ (scale * inp[f"{p}_dw_b"] + shift) + inp[f"{p}_pw_b"]
    ).astype(np.float32)
    return w9t, const.reshape(C, 1)


def kernel(**inputs):
    inp = {k: np.asarray(v) for k, v in inputs.items()}
    x = inp["x"].astype(np.float32)          # (8, 3136, 256)
    B = x.shape[0]

    if "nc" not in _CACHED:
        _CACHED["nc"] = _build_nc()
    nc = _CACHED["nc"]

    common = {}
    for p in ("q", "k", "v"):
        w9t, const = _fold_weights(inp, p)
        common[f"{p}_w9t"] = w9t
        common[f"{p}_const"] = const
    common["proj_wt"] = np.ascontiguousarray(inp["proj_w"].T).astype(ml_dtypes.bfloat16)
    common["proj_bv"] = inp["proj_b"].reshape(C, 1).astype(np.float32)

    xb = x.astype(ml_dtypes.bfloat16)
    in_maps = [dict(common, x=np.ascontiguousarray(xb[b])) for b in range(B)]
    res = run_bass_kernel_spmd(nc, in_maps, list(range(B)))
    out = np.stack([res.results[b]["out"] for b in range(B)], axis=0)
    return out.astype(np.float32)
